# revision 83
# baseline (speedup 1.0000x reference)
"""Trainium2 Bass kernel for nn_Net_60052232733176 (gnn_message_passing).

Graph-data parallel over 8 cores (7 graphs max per core). Dense per-graph
formulation, feat-major (h^T) primary layout:

  - Host re-encodes edges as a dense bf16 multiplicity matrix cnt[1024,1024]
    with +1 on the diagonal (GAT self-loops); GraphConv subtracts the diag
    term back out with one tensor op.
  - GAT: rank-1 logits exp(lrelu(es_u+ed_v)) masked by cnt; attention
    aggregation and denominators via PE matmuls over a bf16 pairwise tile;
    normalization as a single tensor divide in feat-major space.  Dead
    nodes are handled by a -100 fold on es (exp -> 0) and by pool
    coefficients being exactly 0 (no explicit mask multiplies).
  - TopK pooling: scores via f32r matmuls from the f32 feat-major state;
    rank counting with an fp16 compare tile + ones-matmul column sums;
    pool/mask coefficient rows broadcast to all partitions with gpsimd
    partition_broadcast; readouts are free-axis reduces in feat-major.
  - Final MLP batched over graphs, log-softmax via a Newton iteration for
    ln (keeps everything on the single resident ACT table).

Self-contained: hardcodes all shapes; no file reads.
"""
import os
import numpy as np

import concourse.bass as bass
import concourse.bacc as bacc
import concourse.mybir as mybir
import concourse.tile as tile
from concourse.bass_utils import run_bass_kernel_spmd
from concourse.masks import make_identity

F32 = mybir.dt.float32
F32R = mybir.dt.float32r
BF16 = mybir.dt.bfloat16
FP16 = mybir.dt.float16
AF = mybir.ActivationFunctionType
OP = mybir.AluOpType
AX = mybir.AxisListType

P = 128
B, NPG, D, C = 50, 1000, 128, 10
NP_ = 1024
NT = NP_ // P          # 8 node chunks
NCORES = 8
G = 7                  # graph slots per core
K1, K2, K3 = 800, 640, 512
BIGM = 100.0           # dead-node fold on es before exp
BIGS = 30000.0         # dead-node fold for topk scores (fp16-safe)

_cache = {}

CH = [slice(c * P, (c + 1) * P) for c in range(NT)]


def _build_program():
    KG = int(os.environ.get("K_GRAPHS", G))
    nc = bacc.Bacc(None, target_bir_lowering=False)

    # ---- DRAM tensors ----
    cnt_d = nc.dram_tensor("cnt_sh", [G, P, NT * NP_], BF16, kind="ExternalInput")
    xT_d = nc.dram_tensor("xT_sh", [G, P, NP_], BF16, kind="ExternalInput")
    xnm_d = nc.dram_tensor("xnm_sh", [G, P, NP_], BF16, kind="ExternalInput")
    mfM0_d = nc.dram_tensor("mfM0", [P, NT], F32, kind="ExternalInput")
    mfS0_d = nc.dram_tensor("mfS0", [33, NP_], FP16, kind="ExternalInput")

    def wparam(name, shape, dtype=F32):
        return nc.dram_tensor(name, shape, dtype, kind="ExternalInput")

    Wg_d = [wparam(f"W_g{l}", [D, D], BF16) for l in (1, 2, 3)]
    asd_d = [wparam(f"asd_g{l}", [D, 2], BF16) for l in (1, 2, 3)]
    bg_d = [wparam(f"b_g{l}", [D, 1]) for l in (1, 2, 3)]
    Wr_d = [wparam(f"Wr_c{l}", [D, D], BF16) for l in (1, 2, 3)]
    br_d = [wparam(f"br_c{l}", [D, 1]) for l in (1, 2, 3)]
    Wo_d = [wparam(f"Wo_c{l}", [D, D], BF16) for l in (1, 2, 3)]
    wp_d = {n: wparam(n, [D, 1], BF16) for n in ("w_p20", "w_p30", "w_p11", "w_p21", "w_p31")}
    Wl1_d = wparam("W_l1", [2 * D, D], BF16)
    bl1_d = wparam("b_l1", [D, 1])
    Wl2_d = wparam("W_l2", [D, 64], BF16)
    bl2_d = wparam("b_l2", [64, 1])
    Wl3_d = wparam("W_l3", [64, C], BF16)
    bl3_d = wparam("b_l3", [C, 1])

    out_d = nc.dram_tensor("out", [G, C], F32, kind="ExternalOutput")

    with tile.TileContext(nc) as tc:
        import contextlib
        with contextlib.ExitStack() as ctx:
            cp = ctx.enter_context(tc.tile_pool(name="const", bufs=1))
            dp = ctx.enter_context(tc.tile_pool(name="dma", bufs=2))
            dc = ctx.enter_context(tc.tile_pool(name="dmac", bufs=3))
            Lp = ctx.enter_context(tc.tile_pool(name="Ltile", bufs=2))
            Gp = ctx.enter_context(tc.tile_pool(name="Gm", bufs=2))
            sp = ctx.enter_context(tc.tile_pool(name="state", bufs=3))
            s3 = ctx.enter_context(tc.tile_pool(name="state3", bufs=4))
            rp = ctx.enter_context(tc.tile_pool(name="reps", bufs=2))
            rp1 = ctx.enter_context(tc.tile_pool(name="reps1", bufs=1))
            vp = ctx.enter_context(tc.tile_pool(name="vec", bufs=1))
            psW = ctx.enter_context(tc.tile_pool(name="psW", bufs=2, space="PSUM"))
            psG = ctx.enter_context(tc.tile_pool(name="psG", bufs=1, space="PSUM"))
            psR = ctx.enter_context(tc.tile_pool(name="psR", bufs=3, space="PSUM"))
            psS = ctx.enter_context(tc.tile_pool(name="psS", bufs=1, space="PSUM"))

            # ---- constants ----
            ident = cp.tile([P, P], F32, tag="ident")
            make_identity(nc, ident[:])
            ident_h = cp.tile([P, P], FP16, tag="identh")
            nc.vector.tensor_copy(out=ident_h[:], in_=ident[:])
            ones_bf = cp.tile([P, 1], BF16, tag="onesbf")
            nc.vector.memset(ones_bf[:], 1.0)
            ones_f16 = cp.tile([P, 1], FP16, tag="onesf16")
            nc.vector.memset(ones_f16[:], 1.0)
            # PE warm-up: absorb gpsimd identity dep
            warm = psS.tile([P, 16], F32, tag="pcol")
            nc.tensor.matmul(warm[0:16, :], ident[:, 0:16], ident[:, 0:16],
                             start=True, stop=True)

            def load_w(dram, shape, tag, dtype=F32):
                t = cp.tile(shape, dtype, tag=tag)
                nc.sync.dma_start(out=t[:], in_=dram[:])
                return t

            Wg = [load_w(Wg_d[i], [D, D], f"Wg{i}", BF16) for i in range(3)]
            asd = [load_w(asd_d[i], [D, 2], f"asd{i}", BF16) for i in range(3)]
            bg = [load_w(bg_d[i], [D, 1], f"bg{i}") for i in range(3)]
            Wr = [load_w(Wr_d[i], [D, D], f"Wr{i}", BF16) for i in range(3)]
            br = [load_w(br_d[i], [D, 1], f"br{i}") for i in range(3)]
            Wo = [load_w(Wo_d[i], [D, D], f"Wo{i}", BF16) for i in range(3)]
            wp = {n: load_w(d, [D, 1], n, BF16) for n, d in wp_d.items()}
            Wl1a = cp.tile([D, D], BF16, tag="Wl1a")
            nc.sync.dma_start(out=Wl1a[:], in_=Wl1_d[0:D, :])
            Wl1b = cp.tile([D, D], BF16, tag="Wl1b")
            nc.sync.dma_start(out=Wl1b[:], in_=Wl1_d[D:2 * D, :])
            bl1 = load_w(bl1_d, [D, 1], "bl1")
            Wl2 = load_w(Wl2_d, [D, 64], "Wl2", BF16)
            bl2 = load_w(bl2_d, [64, 1], "bl2")
            Wl3 = load_w(Wl3_d, [64, C], "Wl3", BF16)
            bl3 = load_w(bl3_d, [C, 1], "bl3")
            mfM0 = load_w(mfM0_d, [P, NT], "mfM0")
            mfS0 = cp.tile([64, NP_], FP16, tag="mfS0")
            nc.sync.dma_start(out=mfS0[0:33, :], in_=mfS0_d[:])

            gacc0 = []
            gacc1 = []
            for g in range(G):
                ga = cp.tile([P, 1], BF16, tag=f"gacc0_{g}")
                gb = cp.tile([P, 1], BF16, tag=f"gacc1_{g}")
                gacc0.append(ga)
                gacc1.append(gb)
            for g in range(G):
                nc.vector.memset(gacc0[g][:], 0.0)
                nc.vector.memset(gacc1[g][:], 0.0)

            pools_gat = [("w_p20", K1), ("w_p20", K2), ("w_p30", K3)]
            pools_gc = [("w_p11", K1), ("w_p21", K2), ("w_p31", K3)]

            def s_mm(out_row, w_col, hT):
                """scores row: out[0,v] = sum_d w[d]*hT[d,v] (f32r moving)."""
                for h in range(2):
                    sl = slice(h * 512, (h + 1) * 512)
                    nc.tensor.matmul(out_row[:, sl], w_col[:], hT[:, sl],
                                     start=True, stop=True)

            def gat_phase1(li, hTb, mfM):
                """input hTb bf16 feat-major; returns hTf_next f32 feat-major."""
                # est cols + hW node-major, sharing each hTb chunk stationary;
                # hW psum in two half-tiles (1 bank each)
                est_ps = psS.tile([P, 2 * NT], F32, tag="pcol")
                hw_h = [psW.tile([P, 512], F32, tag="gatW", name=f"hw{q}")
                        for q in range(2)]
                for c in range(NT):
                    nc.tensor.matmul(est_ps[:, 2 * c:2 * c + 2], hTb[:, CH[c]],
                                     asd[li][:], start=True, stop=True)
                    nc.tensor.matmul(hw_h[c // 4][:, (c % 4) * P:(c % 4 + 1) * P],
                                     hTb[:, CH[c]], Wg[li][:],
                                     start=True, stop=True)
                est = vp.tile([P, 2 * NT], F32, tag="est")
                nc.vector.tensor_copy(out=est[:], in_=est_ps[:])
                es2 = vp.tile([P, NT], F32, tag="es2")
                nc.vector.tensor_tensor(
                    out=es2[:], in0=est[:].rearrange("p (c two) -> p c two", two=2)[:, :, 0],
                    in1=mfM[:], op=OP.add)
                hW_bf = sp.tile([P, NP_], BF16, tag="hWbf")
                for q in range(2):
                    nc.vector.tensor_copy(out=hW_bf[:, q * 512:(q + 1) * 512],
                                          in_=hw_h[q][:])
                edv = est[:].rearrange("p (c two) -> p c two", two=2)
                ed_rep = sp.tile([P, NP_], BF16, tag="edrep")
                for q in range(2):
                    ed_ps = psW.tile([P, 512], F32, tag="gatW")
                    for j in range(4):
                        c = q * 4 + j
                        nc.tensor.matmul(ed_ps[:, j * P:(j + 1) * P],
                                         edv[:, c, 1:2].to_broadcast([P, P]),
                                         ident[:], is_transpose=True)
                    nc.scalar.activation(out=ed_rep[:, q * 512:(q + 1) * 512],
                                         in_=ed_ps[:], func=AF.Copy)
                return es2, hW_bf, ed_rep

            def gat_phase2(li, cnt_bf, es2, hW_bf, ed_rep):
                # pairwise tile; agg half 0 rides the L pipeline, half 1 after
                L = Lp.tile([P, NT * NP_], BF16, tag="L")
                agg_h = [psW.tile([P, 512], F32, tag="gatW", name=f"agg{q}")
                         for q in range(2)]
                drow_h = [psR.tile([1, 512], F32, tag="rows", name=f"drow{q}")
                          for q in range(2)]
                for t in range(NT):
                    sl = slice(t * NP_, (t + 1) * NP_)
                    nc.scalar.activation(out=L[:, sl], in_=ed_rep[:], func=AF.Prelu,
                                         alpha=0.2, bias=es2[:, t:t + 1])
                    nc.scalar.activation(out=L[:, sl], in_=L[:, sl], func=AF.Exp)
                    nc.vector.tensor_tensor(out=L[:, sl], in0=L[:, sl],
                                            in1=cnt_bf[:, sl], op=OP.mult)
                    hs0 = slice(t * NP_, t * NP_ + 512)
                    nc.tensor.matmul(agg_h[0][:], hW_bf[:, CH[t]], L[:, hs0],
                                     start=(t == 0), stop=(t == NT - 1))
                    nc.tensor.matmul(drow_h[0][:], ones_bf[:], L[:, hs0],
                                     start=(t == 0), stop=(t == NT - 1))
                for t in range(NT):
                    hs1 = slice(t * NP_ + 512, (t + 1) * NP_)
                    nc.tensor.matmul(agg_h[1][:], hW_bf[:, CH[t]], L[:, hs1],
                                     start=(t == 0), stop=(t == NT - 1))
                    nc.tensor.matmul(drow_h[1][:], ones_bf[:], L[:, hs1],
                                     start=(t == 0), stop=(t == NT - 1))
                # normalize + bias + relu, per half
                hTf = s3.tile([P, NP_], BF16, tag="hT")
                for hh in range(2):
                    hsl = slice(hh * 512, (hh + 1) * 512)
                    rd = vp.tile([1, 512], F32, tag=f"den{hh}")
                    nc.vector.reciprocal(out=rd[:], in_=drow_h[hh][0:1, :])
                    rd_rep = rp1.tile([P, 512], F32, tag=f"denrep{hh}")
                    nc.gpsimd.partition_broadcast(rd_rep[:], rd[0:1, :])
                    nc.vector.tensor_tensor(out=hTf[:, hsl], in0=agg_h[hh][:],
                                            in1=rd_rep[:], op=OP.mult)
                    nc.scalar.activation(out=hTf[:, hsl], in_=hTf[:, hsl],
                                         func=AF.Relu, bias=bg[li][:, 0:1])
                return hTf

            def gc_agg(li, cnt_bf, znm_b, zT_sub):
                """GraphConv aggregate (PE + 2 subtract TTs only)."""
                aggT_bf = sp.tile([P, NP_], BF16, tag="aggTbf")
                for hh in range(2):
                    hsl = slice(hh * 512, (hh + 1) * 512)
                    agg_ps = psG.tile([P, 512], F32, tag="gcW")
                    for t in range(NT):
                        nc.tensor.matmul(
                            agg_ps[:], znm_b[:, CH[t]],
                            cnt_bf[:, t * NP_ + hh * 512: t * NP_ + (hh + 1) * 512],
                            start=(t == 0), stop=(t == NT - 1))
                    # subtract the +I diag contribution (no self-loop in GraphConv)
                    nc.vector.tensor_tensor(out=aggT_bf[:, hsl], in0=agg_ps[:],
                                            in1=zT_sub[:, hsl], op=OP.subtract)
                return aggT_bf

            def gc_out(li, aggT_bf, zTb):
                zTf = s3.tile([P, NP_], BF16, tag="zT")
                for hh in range(2):
                    hsl = slice(hh * 512, (hh + 1) * 512)
                    out_ps = psG.tile([P, 512], F32, tag="gcW")
                    nc.tensor.matmul(out_ps[:], Wr[li][:], aggT_bf[:, hsl],
                                     start=True, stop=False)
                    nc.tensor.matmul(out_ps[:], Wo[li][:], zTb[:, hsl],
                                     start=False, stop=True)
                    nc.scalar.activation(out=zTf[:, hsl], in_=out_ps[:],
                                         func=AF.Relu, bias=br[li][:, 0:1])
                return zTf

            def pair(t2):
                """[64, N] tile -> partitions 0..32 slice (rows 0 and 32 carry
                the two branches; rows 1..31 are don't-care)."""
                return t2[0:33, :]

            def topk_branch(g, li, br_tag, hTf, mfS, w_col, k, out_tile, need_col):
                """topk+readout for one branch. Writes pooled state into
                out_tile; returns (mfS_next, mfM_next_or_None)."""
                sm = vp.tile([1, NP_], FP16, tag=f"sm_{br_tag}")
                th = vp.tile([1, NP_], FP16, tag=f"th_{br_tag}")
                for hh in range(2):
                    hsl = slice(hh * 512, (hh + 1) * 512)
                    s_ps = psR.tile([1, 512], F32, tag="rows", name=f"sps{hh}")
                    nc.tensor.matmul(s_ps[:], w_col[:], hTf[:, hsl],
                                     start=True, stop=True)
                    nc.vector.tensor_tensor(out=sm[0:1, hsl], in0=s_ps[:],
                                            in1=mfS[0:1, hsl], op=OP.add)
                    nc.scalar.activation(out=th[0:1, hsl], in_=sm[0:1, hsl],
                                         func=AF.Tanh)
                # masked-score columns (compare scalars): smc[p,c]=sm[c*128+p]
                smc_ps = psS.tile([P, 2 * NT], FP16, tag="pcol2")
                for c in range(NT):
                    nc.tensor.matmul(smc_ps[:, 2 * c:2 * c + 1], sm[0:1, CH[c]],
                                     ident_h[0:1, 0:1], is_transpose=True)
                smc = vp.tile([P, NT], F32, tag=f"smc_{br_tag}")
                nc.vector.tensor_copy(
                    out=smc[:],
                    in_=smc_ps[:].rearrange("p (c two) -> p c two", two=2)[:, :, 0])
                srep = rp.tile([P, NP_], FP16, tag=f"srep_{br_tag}")
                nc.gpsimd.partition_broadcast(srep[:], sm[0:1, :])
                rr_h = [psR.tile([1, 512], F32, tag="rows", name=f"rr{q}")
                        for q in range(2)]
                for t2 in range(NT // 2):
                    Gm = Gp.tile([P, 2 * NP_], FP16, tag=f"Gm_{br_tag}")
                    for j in range(2):
                        t = 2 * t2 + j
                        nc.vector.tensor_scalar(
                            out=Gm[:, j * NP_:(j + 1) * NP_], in0=srep[:],
                            scalar1=smc[:, t:t + 1], scalar2=None, op0=OP.is_lt)
                        for hh in range(2):
                            nc.tensor.matmul(
                                rr_h[hh][:], ones_f16[:],
                                Gm[:, j * NP_ + hh * 512:j * NP_ + (hh + 1) * 512],
                                start=(t == 0), stop=(t == NT - 1))
                # pool = tanh(s) * (rank < k) fused; mfS from pool == 0
                pool = vp.tile([1, NP_], FP16, tag=f"pool_{br_tag}")
                for hh in range(2):
                    hsl = slice(hh * 512, (hh + 1) * 512)
                    nc.vector.scalar_tensor_tensor(
                        out=pool[0:1, hsl], in0=rr_h[hh][0:1, :],
                        scalar=float(k), in1=th[0:1, hsl], op0=OP.is_lt,
                        op1=OP.mult)
                mfS_next = vp.tile([1, NP_], FP16, tag=f"mfS_{br_tag}")
                nc.vector.tensor_scalar(out=mfS_next[:], in0=pool[:], scalar1=0.0,
                                        scalar2=-BIGS, op0=OP.is_equal, op1=OP.mult)
                mfM_next = None
                if need_col:
                    kc_ps = psS.tile([P, 2 * NT], FP16, tag="pcol2")
                    for c in range(NT):
                        nc.tensor.matmul(kc_ps[:, 2 * c:2 * c + 1],
                                         mfS_next[0:1, CH[c]],
                                         ident_h[0:1, 0:1], is_transpose=True)
                    mfM_next = vp.tile([P, NT], F32, tag="mfM")
                    nc.vector.tensor_scalar(
                        out=mfM_next[:],
                        in0=kc_ps[:].rearrange("p (c two) -> p c two", two=2)[:, :, 0],
                        scalar1=BIGM / BIGS, scalar2=None, op0=OP.mult)
                prep = rp.tile([P, NP_], FP16, tag=f"prep_{br_tag}")
                nc.gpsimd.partition_broadcast(prep[:], pool[0:1, :])
                mrep = rp.tile([P, NP_], FP16, tag=f"mrep_{br_tag}")
                nc.gpsimd.partition_broadcast(mrep[:], mfS_next[0:1, :])
                nc.vector.tensor_tensor(out=out_tile[:], in0=hTf[:], in1=prep[:],
                                        op=OP.mult)
                hm = rp1.tile([P, NP_], BF16, tag=f"hm_{br_tag}")
                nc.vector.tensor_tensor(out=hm[:], in0=out_tile[:], in1=mrep[:],
                                        op=OP.add)
                mx = vp.tile([P, 1], BF16, tag="mx")
                nc.vector.tensor_reduce(out=mx[:], in_=hm[:], axis=AX.X, op=OP.max)
                nc.vector.tensor_tensor(out=gacc0[g][:], in0=gacc0[g][:],
                                        in1=mx[:], op=OP.add)
                mn = vp.tile([P, 1], BF16, tag="mn")
                with nc.allow_low_precision(reason="bf16 mean readout"):
                    nc.vector.tensor_reduce(out=mn[:], in_=out_tile[:], axis=AX.X,
                                            op=OP.add)
                nc.vector.tensor_scalar(out=mn[:], in0=mn[:], scalar1=1.0 / k,
                                        scalar2=None, op0=OP.mult)
                nc.vector.tensor_tensor(out=gacc1[g][:], in0=gacc1[g][:],
                                        in1=mn[:], op=OP.add)
                return mfS_next, mfM_next

            # ---- per-graph loop: wavefront schedule ----
            SKEW = int(os.environ.get("K_SKEW", 2))
            st = {}

            def load_graph(g):
                cnt_bf = dc.tile([P, NT * NP_], BF16, tag="cnt")
                for q in range(4):
                    qs = slice(q * 2 * NP_, (q + 1) * 2 * NP_)
                    nc.sync.dma_start(out=cnt_bf[:, qs], in_=cnt_d[g][:, qs])
                xT_b = dp.tile([P, NP_], BF16, tag="xT")
                nc.sync.dma_start(out=xT_b[:], in_=xT_d[g][:])
                xnm_b = dp.tile([P, NP_], BF16, tag="xnm")
                nc.sync.dma_start(out=xnm_b[:], in_=xnm_d[g][:])
                st[g] = dict(cnt=cnt_bf, hTb=xT_b, znm=xnm_b, zTb=xT_b,
                             zsub=xT_b, mfM=mfM0, mfS_g=mfS0, mfS_c=mfS0)

            def do_level(g, li):
                S = st[g]
                last = (li == 2)
                k = pools_gat[li][1]
                NO_GAT = os.environ.get("K_NO_GAT") == "1"
                NO_GC = os.environ.get("K_NO_GC") == "1"
                if not NO_GAT:
                    es2, hW_bf, ed_rep = gat_phase1(li, S["hTb"], S["mfM"])
                if not NO_GC:
                    aggT_bf = gc_agg(li, S["cnt"], S["znm"], S["zsub"])
                hTf = (gat_phase2(li, S["cnt"], es2, hW_bf, ed_rep)
                       if not NO_GAT else None)
                zTf = gc_out(li, aggT_bf, S["zTb"]) if not NO_GC else hTf
                if hTf is None:
                    hTf = zTf
                h_poolT = s3.tile([P, NP_], BF16, tag="hT")
                S["mfS_g"], mfM_n = topk_branch(
                    g, li, "g", hTf, S["mfS_g"], wp[pools_gat[li][0]], k,
                    h_poolT, need_col=not last)
                if mfM_n is not None:
                    S["mfM"] = mfM_n
                z_poolT = s3.tile([P, NP_], BF16, tag="zT")
                S["mfS_c"], _ = topk_branch(
                    g, li, "c", zTf, S["mfS_c"], wp[pools_gc[li][0]], k,
                    z_poolT, need_col=False)
                S["zsub"] = z_poolT
                if not last:
                    S["hTb"] = h_poolT
                    S["zTb"] = z_poolT
                    znm_b = sp.tile([P, NP_], BF16, tag="znm")
                    nc.sync.dma_start_transpose(
                        znm_b[:].rearrange("p (c d) -> p c d", d=P), z_poolT[:])
                    S["znm"] = znm_b

            events = []  # (wave, order, kind, g, li)
            for g in range(KG):
                events.append((g * SKEW - 1, 0, "load", g, 0))
                for li in range(3):
                    events.append((g * SKEW + li, 1, "level", g, li))
            events.sort(key=lambda e: (e[0], e[1], e[3]))
            for _, _, kind, g, li in events:
                if kind == "load":
                    load_graph(g)
                else:
                    do_level(g, li)

            # ---- MLP over all graphs ----
            t1_ps = psS.tile([P, 16], F32, tag="pcol")
            for g in range(G):
                nc.tensor.matmul(t1_ps[:, g:g + 1], Wl1a[:], gacc0[g][:],
                                 start=True, stop=False)
                nc.tensor.matmul(t1_ps[:, g:g + 1], Wl1b[:], gacc1[g][:],
                                 start=False, stop=True)
            t1 = cp.tile([P, G], BF16, tag="t1")
            nc.vector.tensor_scalar(out=t1[:], in0=t1_ps[:, 0:G], scalar1=bl1[:, 0:1],
                                    scalar2=0.0, op0=OP.add, op1=OP.max)
            t2_ps = psS.tile([P, 16], F32, tag="pcol")
            nc.tensor.matmul(t2_ps[0:64, 0:G], Wl2[:], t1[:], start=True, stop=True)
            t2p = cp.tile([64, G], BF16, tag="t2p")
            nc.vector.tensor_scalar(out=t2p[:], in0=t2_ps[0:64, 0:G], scalar1=bl2[:, 0:1],
                                    scalar2=None, op0=OP.add)
            t2 = cp.tile([64, G], BF16, tag="t2")
            nc.scalar.activation(out=t2[:], in_=t2p[:], func=AF.Prelu, alpha=0.01)
            t3_ps = psS.tile([P, 16], F32, tag="pcol")
            nc.tensor.matmul(t3_ps[0:C, 0:G], Wl3[:], t2[:], start=True, stop=True)
            lg_cm = cp.tile([C, G], F32, tag="lgcm")
            nc.vector.tensor_scalar(out=lg_cm[:], in0=t3_ps[0:C, 0:G], scalar1=bl3[:, 0:1],
                                    scalar2=None, op0=OP.add)
            lg_ps = psS.tile([P, 16], F32, tag="pcol")
            nc.tensor.matmul(lg_ps[0:G, 0:C], lg_cm[:], ident[0:C, 0:C],
                             is_transpose=True)
            lg = cp.tile([G, C], F32, tag="lg")
            nc.vector.tensor_copy(out=lg[:], in_=lg_ps[0:G, 0:C])
            ex = cp.tile([G, C], F32, tag="ex")
            nc.scalar.activation(out=ex[:], in_=lg[:], func=AF.Exp)
            S = cp.tile([G, 1], F32, tag="S")
            nc.vector.tensor_reduce(out=S[:], in_=ex[:], axis=AX.X, op=OP.add)
            # ln(S) via Newton: y += S*exp(-y) - 1
            y = cp.tile([G, 1], F32, tag="y")
            nc.vector.memset(y[:], 2.3)
            for _ in range(6):
                eny = cp.tile([G, 1], F32, tag="eny")
                nc.scalar.activation(out=eny[:], in_=y[:], func=AF.Exp, scale=-1.0)
                nc.vector.tensor_tensor(out=eny[:], in0=eny[:], in1=S[:], op=OP.mult)
                nc.vector.tensor_scalar(out=eny[:], in0=eny[:], scalar1=1.0,
                                        scalar2=None, op0=OP.subtract)
                nc.vector.tensor_tensor(out=y[:], in0=y[:], in1=eny[:], op=OP.add)
            outt = cp.tile([G, C], F32, tag="outt")
            nc.vector.tensor_scalar(out=outt[:], in0=lg[:], scalar1=y[:, 0:1],
                                    scalar2=None, op0=OP.subtract)
            nc.sync.dma_start(out=out_d[:], in_=outt[:])

    nc.compile()
    return nc


# ----------------------------------------------------------------------------
# host side
# ----------------------------------------------------------------------------

def _prep_in_maps(inputs):
    import ml_dtypes
    bf16 = ml_dtypes.bfloat16
    x = np.ascontiguousarray(np.asarray(inputs["x"], np.float32))
    ei = np.asarray(inputs["edge_index"]).astype(np.int64)
    src, dst = ei[0], ei[1]
    gid = src // NPG
    sl, dl = src % NPG, dst % NPG

    cnt = np.zeros((B, NP_, NP_), np.float32)
    np.add.at(cnt, (gid, sl, dl), 1.0)
    idx = np.arange(NP_)
    cnt[:, idx, idx] += 1.0  # GAT self-loops (GraphConv subtracts this back)
    # pack [g, src, dst] -> [g, p, t*1024+dst]
    cnt_pk = np.ascontiguousarray(
        cnt.reshape(B, NT, P, NP_).transpose(0, 2, 1, 3).reshape(B, P, NT * NP_)
    ).astype(bf16)

    x3 = x.reshape(B, NPG, D)
    x_pad = np.zeros((B, NP_, D), np.float32)
    x_pad[:, :NPG] = x3
    xT = np.ascontiguousarray(x_pad.transpose(0, 2, 1)).astype(bf16)  # [B,128,1024]
    xnm = np.ascontiguousarray(
        x_pad.reshape(B, NT, P, D).transpose(0, 2, 1, 3).reshape(B, P, NP_)
    ).astype(bf16)

    m0 = np.zeros((NP_,), np.float32)
    m0[:NPG] = 1.0
    mfM0 = np.ascontiguousarray(((m0 - 1.0) * BIGM).reshape(NT, P).T)  # [128, 8]
    mfS0 = np.zeros((33, NP_), np.float16)
    mfS0[0] = mfS0[32] = ((m0 - 1.0) * BIGS).astype(np.float16)

    def col(v):
        return np.ascontiguousarray(np.asarray(v, np.float32).reshape(-1, 1))

    weights = {}
    for l in (1, 2, 3):
        Wgl = np.asarray(inputs[f"W_g{l}"], np.float32)
        weights[f"W_g{l}"] = np.ascontiguousarray(Wgl).astype(bf16)
        weights[f"asd_g{l}"] = np.ascontiguousarray(
            Wgl @ np.stack([np.asarray(inputs[f"as_g{l}"], np.float32),
                            np.asarray(inputs[f"ad_g{l}"], np.float32)], axis=1)
        ).astype(bf16)
        weights[f"b_g{l}"] = col(inputs[f"b_g{l}"])
        weights[f"Wr_c{l}"] = np.ascontiguousarray(
            np.asarray(inputs[f"Wr_c{l}"], np.float32)).astype(bf16)
        weights[f"br_c{l}"] = col(inputs[f"br_c{l}"])
        weights[f"Wo_c{l}"] = np.ascontiguousarray(
            np.asarray(inputs[f"Wo_c{l}"], np.float32)).astype(bf16)
    for n in ("w_p20", "w_p30", "w_p11", "w_p21", "w_p31"):
        w = np.asarray(inputs[n], np.float32)
        weights[n] = col(w / np.linalg.norm(w)).astype(bf16)
    weights["W_l1"] = np.ascontiguousarray(np.asarray(inputs["W_l1"], np.float32)).astype(bf16)
    weights["b_l1"] = col(inputs["b_l1"])
    weights["W_l2"] = np.ascontiguousarray(np.asarray(inputs["W_l2"], np.float32)).astype(bf16)
    weights["b_l2"] = col(inputs["b_l2"])
    weights["W_l3"] = np.ascontiguousarray(np.asarray(inputs["W_l3"], np.float32)).astype(bf16)
    weights["b_l3"] = col(inputs["b_l3"])

    in_maps = []
    for c in range(NCORES):
        lo = c * G
        hi = min(lo + G, B)
        cs = np.zeros((G, P, NT * NP_), bf16)
        xs = np.zeros((G, P, NP_), bf16)
        xn = np.zeros((G, P, NP_), bf16)
        if hi > lo:
            cs[:hi - lo] = cnt_pk[lo:hi]
            xs[:hi - lo] = xT[lo:hi]
            xn[:hi - lo] = xnm[lo:hi]
        im = {"cnt_sh": cs, "xT_sh": xs, "xnm_sh": xn,
              "mfM0": mfM0, "mfS0": mfS0}
        im.update(weights)
        in_maps.append(im)
    return in_maps


def kernel(**inputs) -> np.ndarray:
    if "nc" not in _cache:
        _cache["nc"] = _build_program()
    nc = _cache["nc"]
    in_maps = _prep_in_maps(inputs)
    res = run_bass_kernel_spmd(nc, in_maps, list(range(NCORES)))
    out = np.zeros((B, C), np.float32)
    for c in range(NCORES):
        lo = c * G
        hi = min(lo + G, B)
        if hi > lo:
            out[lo:hi] = np.asarray(res.results[c]["out"])[:hi - lo]
    return out


# revision 84
# speedup vs baseline: 1.0005x; 1.0005x over previous
"""Trainium2 Bass kernel for nn_Net_60052232733176 (gnn_message_passing).

Graph-data parallel over 8 cores (7 graphs max per core). Dense per-graph
formulation, feat-major (h^T) primary layout:

  - Host re-encodes edges as a dense bf16 multiplicity matrix cnt[1024,1024]
    with +1 on the diagonal (GAT self-loops); GraphConv subtracts the diag
    term back out with one tensor op.
  - GAT: rank-1 logits exp(lrelu(es_u+ed_v)) masked by cnt; attention
    aggregation and denominators via PE matmuls over a bf16 pairwise tile;
    normalization as a single tensor divide in feat-major space.  Dead
    nodes are handled by a -100 fold on es (exp -> 0) and by pool
    coefficients being exactly 0 (no explicit mask multiplies).
  - TopK pooling: scores via f32r matmuls from the f32 feat-major state;
    rank counting with an fp16 compare tile + ones-matmul column sums;
    pool/mask coefficient rows broadcast to all partitions with gpsimd
    partition_broadcast; readouts are free-axis reduces in feat-major.
  - Final MLP batched over graphs, log-softmax via a Newton iteration for
    ln (keeps everything on the single resident ACT table).

Self-contained: hardcodes all shapes; no file reads.
"""
import os
import numpy as np

import concourse.bass as bass
import concourse.bacc as bacc
import concourse.mybir as mybir
import concourse.tile as tile
from concourse.bass_utils import run_bass_kernel_spmd
from concourse.masks import make_identity

F32 = mybir.dt.float32
F32R = mybir.dt.float32r
BF16 = mybir.dt.bfloat16
FP16 = mybir.dt.float16
AF = mybir.ActivationFunctionType
OP = mybir.AluOpType
AX = mybir.AxisListType

P = 128
B, NPG, D, C = 50, 1000, 128, 10
NP_ = 1024
NT = NP_ // P          # 8 node chunks
NCORES = 8
G = 7                  # graph slots per core
K1, K2, K3 = 800, 640, 512
BIGM = 100.0           # dead-node fold on es before exp
BIGS = 30000.0         # dead-node fold for topk scores (fp16-safe)

_cache = {}

CH = [slice(c * P, (c + 1) * P) for c in range(NT)]


def _build_program():
    KG = int(os.environ.get("K_GRAPHS", G))
    nc = bacc.Bacc(None, target_bir_lowering=False)

    # ---- DRAM tensors ----
    cnt_d = nc.dram_tensor("cnt_sh", [G, P, NT * NP_], BF16, kind="ExternalInput")
    xT_d = nc.dram_tensor("xT_sh", [G, P, NP_], BF16, kind="ExternalInput")
    xnm_d = nc.dram_tensor("xnm_sh", [G, P, NP_], BF16, kind="ExternalInput")
    mfM0_d = nc.dram_tensor("mfM0", [P, NT], F32, kind="ExternalInput")
    mfS0_d = nc.dram_tensor("mfS0", [33, NP_], FP16, kind="ExternalInput")

    def wparam(name, shape, dtype=F32):
        return nc.dram_tensor(name, shape, dtype, kind="ExternalInput")

    Wg_d = [wparam(f"W_g{l}", [D, D], BF16) for l in (1, 2, 3)]
    asd_d = [wparam(f"asd_g{l}", [D, 2], BF16) for l in (1, 2, 3)]
    bg_d = [wparam(f"b_g{l}", [D, 1]) for l in (1, 2, 3)]
    Wr_d = [wparam(f"Wr_c{l}", [D, D], BF16) for l in (1, 2, 3)]
    br_d = [wparam(f"br_c{l}", [D, 1]) for l in (1, 2, 3)]
    Wo_d = [wparam(f"Wo_c{l}", [D, D], BF16) for l in (1, 2, 3)]
    wp_d = {n: wparam(n, [D, 1], BF16) for n in ("w_p20", "w_p30", "w_p11", "w_p21", "w_p31")}
    Wl1_d = wparam("W_l1", [2 * D, D], BF16)
    bl1_d = wparam("b_l1", [D, 1])
    Wl2_d = wparam("W_l2", [D, 64], BF16)
    bl2_d = wparam("b_l2", [64, 1])
    Wl3_d = wparam("W_l3", [64, C], BF16)
    bl3_d = wparam("b_l3", [C, 1])

    out_d = nc.dram_tensor("out", [G, C], F32, kind="ExternalOutput")

    with tile.TileContext(nc) as tc:
        import contextlib
        with contextlib.ExitStack() as ctx:
            cp = ctx.enter_context(tc.tile_pool(name="const", bufs=1))
            dp = ctx.enter_context(tc.tile_pool(name="dma", bufs=2))
            dc = ctx.enter_context(tc.tile_pool(name="dmac", bufs=3))
            Lp = ctx.enter_context(tc.tile_pool(name="Ltile", bufs=2))
            Gp = ctx.enter_context(tc.tile_pool(name="Gm", bufs=3))
            sp = ctx.enter_context(tc.tile_pool(name="state", bufs=3))
            s3 = ctx.enter_context(tc.tile_pool(name="state3", bufs=4))
            rp = ctx.enter_context(tc.tile_pool(name="reps", bufs=2))
            rp1 = ctx.enter_context(tc.tile_pool(name="reps1", bufs=1))
            vp = ctx.enter_context(tc.tile_pool(name="vec", bufs=1))
            psW = ctx.enter_context(tc.tile_pool(name="psW", bufs=2, space="PSUM"))
            psG = ctx.enter_context(tc.tile_pool(name="psG", bufs=1, space="PSUM"))
            psR = ctx.enter_context(tc.tile_pool(name="psR", bufs=3, space="PSUM"))
            psS = ctx.enter_context(tc.tile_pool(name="psS", bufs=1, space="PSUM"))

            # ---- constants ----
            ident = cp.tile([P, P], F32, tag="ident")
            make_identity(nc, ident[:])
            ident_h = cp.tile([P, P], FP16, tag="identh")
            nc.vector.tensor_copy(out=ident_h[:], in_=ident[:])
            ones_bf = cp.tile([P, 1], BF16, tag="onesbf")
            nc.vector.memset(ones_bf[:], 1.0)
            ones_f16 = cp.tile([P, 1], FP16, tag="onesf16")
            nc.vector.memset(ones_f16[:], 1.0)
            # PE warm-up: absorb gpsimd identity dep
            warm = psS.tile([P, 16], F32, tag="pcol")
            nc.tensor.matmul(warm[0:16, :], ident[:, 0:16], ident[:, 0:16],
                             start=True, stop=True)

            def load_w(dram, shape, tag, dtype=F32):
                t = cp.tile(shape, dtype, tag=tag)
                nc.sync.dma_start(out=t[:], in_=dram[:])
                return t

            Wg = [load_w(Wg_d[i], [D, D], f"Wg{i}", BF16) for i in range(3)]
            asd = [load_w(asd_d[i], [D, 2], f"asd{i}", BF16) for i in range(3)]
            bg = [load_w(bg_d[i], [D, 1], f"bg{i}") for i in range(3)]
            Wr = [load_w(Wr_d[i], [D, D], f"Wr{i}", BF16) for i in range(3)]
            br = [load_w(br_d[i], [D, 1], f"br{i}") for i in range(3)]
            Wo = [load_w(Wo_d[i], [D, D], f"Wo{i}", BF16) for i in range(3)]
            wp = {n: load_w(d, [D, 1], n, BF16) for n, d in wp_d.items()}
            Wl1a = cp.tile([D, D], BF16, tag="Wl1a")
            nc.sync.dma_start(out=Wl1a[:], in_=Wl1_d[0:D, :])
            Wl1b = cp.tile([D, D], BF16, tag="Wl1b")
            nc.sync.dma_start(out=Wl1b[:], in_=Wl1_d[D:2 * D, :])
            bl1 = load_w(bl1_d, [D, 1], "bl1")
            Wl2 = load_w(Wl2_d, [D, 64], "Wl2", BF16)
            bl2 = load_w(bl2_d, [64, 1], "bl2")
            Wl3 = load_w(Wl3_d, [64, C], "Wl3", BF16)
            bl3 = load_w(bl3_d, [C, 1], "bl3")
            mfM0 = load_w(mfM0_d, [P, NT], "mfM0")
            mfS0 = cp.tile([64, NP_], FP16, tag="mfS0")
            nc.sync.dma_start(out=mfS0[0:33, :], in_=mfS0_d[:])

            gacc0 = []
            gacc1 = []
            for g in range(G):
                ga = cp.tile([P, 1], BF16, tag=f"gacc0_{g}")
                gb = cp.tile([P, 1], BF16, tag=f"gacc1_{g}")
                gacc0.append(ga)
                gacc1.append(gb)
            for g in range(G):
                nc.vector.memset(gacc0[g][:], 0.0)
                nc.vector.memset(gacc1[g][:], 0.0)

            pools_gat = [("w_p20", K1), ("w_p20", K2), ("w_p30", K3)]
            pools_gc = [("w_p11", K1), ("w_p21", K2), ("w_p31", K3)]

            def s_mm(out_row, w_col, hT):
                """scores row: out[0,v] = sum_d w[d]*hT[d,v] (f32r moving)."""
                for h in range(2):
                    sl = slice(h * 512, (h + 1) * 512)
                    nc.tensor.matmul(out_row[:, sl], w_col[:], hT[:, sl],
                                     start=True, stop=True)

            def gat_phase1(li, hTb, mfM):
                """input hTb bf16 feat-major; returns hTf_next f32 feat-major."""
                # est cols + hW node-major, sharing each hTb chunk stationary;
                # hW psum in two half-tiles (1 bank each)
                est_ps = psS.tile([P, 2 * NT], F32, tag="pcol")
                hw_h = [psW.tile([P, 512], F32, tag="gatW", name=f"hw{q}")
                        for q in range(2)]
                for c in range(NT):
                    nc.tensor.matmul(est_ps[:, 2 * c:2 * c + 2], hTb[:, CH[c]],
                                     asd[li][:], start=True, stop=True)
                    nc.tensor.matmul(hw_h[c // 4][:, (c % 4) * P:(c % 4 + 1) * P],
                                     hTb[:, CH[c]], Wg[li][:],
                                     start=True, stop=True)
                est = vp.tile([P, 2 * NT], F32, tag="est")
                nc.vector.tensor_copy(out=est[:], in_=est_ps[:])
                es2 = vp.tile([P, NT], F32, tag="es2")
                nc.vector.tensor_tensor(
                    out=es2[:], in0=est[:].rearrange("p (c two) -> p c two", two=2)[:, :, 0],
                    in1=mfM[:], op=OP.add)
                hW_bf = sp.tile([P, NP_], BF16, tag="hWbf")
                for q in range(2):
                    nc.vector.tensor_copy(out=hW_bf[:, q * 512:(q + 1) * 512],
                                          in_=hw_h[q][:])
                edv = est[:].rearrange("p (c two) -> p c two", two=2)
                ed_rep = sp.tile([P, NP_], BF16, tag="edrep")
                for q in range(2):
                    ed_ps = psW.tile([P, 512], F32, tag="gatW")
                    for j in range(4):
                        c = q * 4 + j
                        nc.tensor.matmul(ed_ps[:, j * P:(j + 1) * P],
                                         edv[:, c, 1:2].to_broadcast([P, P]),
                                         ident[:], is_transpose=True)
                    nc.scalar.activation(out=ed_rep[:, q * 512:(q + 1) * 512],
                                         in_=ed_ps[:], func=AF.Copy)
                return es2, hW_bf, ed_rep

            def gat_phase2(li, cnt_bf, es2, hW_bf, ed_rep):
                # pairwise tile; agg half 0 rides the L pipeline, half 1 after
                L = Lp.tile([P, NT * NP_], BF16, tag="L")
                agg_h = [psW.tile([P, 512], F32, tag="gatW", name=f"agg{q}")
                         for q in range(2)]
                drow_h = [psR.tile([1, 512], F32, tag="rows", name=f"drow{q}")
                          for q in range(2)]
                for t in range(NT):
                    sl = slice(t * NP_, (t + 1) * NP_)
                    nc.scalar.activation(out=L[:, sl], in_=ed_rep[:], func=AF.Prelu,
                                         alpha=0.2, bias=es2[:, t:t + 1])
                    nc.scalar.activation(out=L[:, sl], in_=L[:, sl], func=AF.Exp)
                    nc.vector.tensor_tensor(out=L[:, sl], in0=L[:, sl],
                                            in1=cnt_bf[:, sl], op=OP.mult)
                    hs0 = slice(t * NP_, t * NP_ + 512)
                    nc.tensor.matmul(agg_h[0][:], hW_bf[:, CH[t]], L[:, hs0],
                                     start=(t == 0), stop=(t == NT - 1))
                    nc.tensor.matmul(drow_h[0][:], ones_bf[:], L[:, hs0],
                                     start=(t == 0), stop=(t == NT - 1))
                for t in range(NT):
                    hs1 = slice(t * NP_ + 512, (t + 1) * NP_)
                    nc.tensor.matmul(agg_h[1][:], hW_bf[:, CH[t]], L[:, hs1],
                                     start=(t == 0), stop=(t == NT - 1))
                    nc.tensor.matmul(drow_h[1][:], ones_bf[:], L[:, hs1],
                                     start=(t == 0), stop=(t == NT - 1))
                # normalize + bias + relu, per half
                hTf = s3.tile([P, NP_], BF16, tag="hT")
                for hh in range(2):
                    hsl = slice(hh * 512, (hh + 1) * 512)
                    rd = vp.tile([1, 512], F32, tag=f"den{hh}")
                    nc.vector.reciprocal(out=rd[:], in_=drow_h[hh][0:1, :])
                    rd_rep = rp1.tile([P, 512], F32, tag=f"denrep{hh}")
                    nc.gpsimd.partition_broadcast(rd_rep[:], rd[0:1, :])
                    nc.vector.tensor_tensor(out=hTf[:, hsl], in0=agg_h[hh][:],
                                            in1=rd_rep[:], op=OP.mult)
                    nc.scalar.activation(out=hTf[:, hsl], in_=hTf[:, hsl],
                                         func=AF.Relu, bias=bg[li][:, 0:1])
                return hTf

            def gc_agg(li, cnt_bf, znm_b, zT_sub):
                """GraphConv aggregate (PE + 2 subtract TTs only)."""
                aggT_bf = sp.tile([P, NP_], BF16, tag="aggTbf")
                for hh in range(2):
                    hsl = slice(hh * 512, (hh + 1) * 512)
                    agg_ps = psG.tile([P, 512], F32, tag="gcW")
                    for t in range(NT):
                        nc.tensor.matmul(
                            agg_ps[:], znm_b[:, CH[t]],
                            cnt_bf[:, t * NP_ + hh * 512: t * NP_ + (hh + 1) * 512],
                            start=(t == 0), stop=(t == NT - 1))
                    # subtract the +I diag contribution (no self-loop in GraphConv)
                    nc.vector.tensor_tensor(out=aggT_bf[:, hsl], in0=agg_ps[:],
                                            in1=zT_sub[:, hsl], op=OP.subtract)
                return aggT_bf

            def gc_out(li, aggT_bf, zTb):
                zTf = s3.tile([P, NP_], BF16, tag="zT")
                for hh in range(2):
                    hsl = slice(hh * 512, (hh + 1) * 512)
                    out_ps = psG.tile([P, 512], F32, tag="gcW")
                    nc.tensor.matmul(out_ps[:], Wr[li][:], aggT_bf[:, hsl],
                                     start=True, stop=False)
                    nc.tensor.matmul(out_ps[:], Wo[li][:], zTb[:, hsl],
                                     start=False, stop=True)
                    nc.scalar.activation(out=zTf[:, hsl], in_=out_ps[:],
                                         func=AF.Relu, bias=br[li][:, 0:1])
                return zTf

            def pair(t2):
                """[64, N] tile -> partitions 0..32 slice (rows 0 and 32 carry
                the two branches; rows 1..31 are don't-care)."""
                return t2[0:33, :]

            def topk_branch(g, li, br_tag, hTf, mfS, w_col, k, out_tile, need_col):
                """topk+readout for one branch. Writes pooled state into
                out_tile; returns (mfS_next, mfM_next_or_None)."""
                sm = vp.tile([1, NP_], FP16, tag=f"sm_{br_tag}")
                th = vp.tile([1, NP_], FP16, tag=f"th_{br_tag}")
                for hh in range(2):
                    hsl = slice(hh * 512, (hh + 1) * 512)
                    s_ps = psR.tile([1, 512], F32, tag="rows", name=f"sps{hh}")
                    nc.tensor.matmul(s_ps[:], w_col[:], hTf[:, hsl],
                                     start=True, stop=True)
                    nc.vector.tensor_tensor(out=sm[0:1, hsl], in0=s_ps[:],
                                            in1=mfS[0:1, hsl], op=OP.add)
                    nc.scalar.activation(out=th[0:1, hsl], in_=sm[0:1, hsl],
                                         func=AF.Tanh)
                # masked-score columns (compare scalars): smc[p,c]=sm[c*128+p]
                smc_ps = psS.tile([P, 2 * NT], FP16, tag="pcol2")
                for c in range(NT):
                    nc.tensor.matmul(smc_ps[:, 2 * c:2 * c + 1], sm[0:1, CH[c]],
                                     ident_h[0:1, 0:1], is_transpose=True)
                smc = vp.tile([P, NT], F32, tag=f"smc_{br_tag}")
                nc.vector.tensor_copy(
                    out=smc[:],
                    in_=smc_ps[:].rearrange("p (c two) -> p c two", two=2)[:, :, 0])
                srep = rp.tile([P, NP_], FP16, tag=f"srep_{br_tag}")
                nc.gpsimd.partition_broadcast(srep[:], sm[0:1, :])
                rr_h = [psR.tile([1, 512], F32, tag="rows", name=f"rr{q}")
                        for q in range(2)]
                for t in range(NT):
                    Gm = Gp.tile([P, NP_], FP16, tag=f"Gm_{br_tag}")
                    nc.vector.tensor_scalar(
                        out=Gm[:], in0=srep[:],
                        scalar1=smc[:, t:t + 1], scalar2=None, op0=OP.is_lt)
                    for hh in range(2):
                        nc.tensor.matmul(rr_h[hh][:], ones_f16[:],
                                         Gm[:, hh * 512:(hh + 1) * 512],
                                         start=(t == 0), stop=(t == NT - 1))
                # pool = tanh(s) * (rank < k) fused; mfS from pool == 0
                pool = vp.tile([1, NP_], FP16, tag=f"pool_{br_tag}")
                for hh in range(2):
                    hsl = slice(hh * 512, (hh + 1) * 512)
                    nc.vector.scalar_tensor_tensor(
                        out=pool[0:1, hsl], in0=rr_h[hh][0:1, :],
                        scalar=float(k), in1=th[0:1, hsl], op0=OP.is_lt,
                        op1=OP.mult)
                mfS_next = vp.tile([1, NP_], FP16, tag=f"mfS_{br_tag}")
                nc.vector.tensor_scalar(out=mfS_next[:], in0=pool[:], scalar1=0.0,
                                        scalar2=-BIGS, op0=OP.is_equal, op1=OP.mult)
                mfM_next = None
                if need_col:
                    kc_ps = psS.tile([P, 2 * NT], FP16, tag="pcol2")
                    for c in range(NT):
                        nc.tensor.matmul(kc_ps[:, 2 * c:2 * c + 1],
                                         mfS_next[0:1, CH[c]],
                                         ident_h[0:1, 0:1], is_transpose=True)
                    mfM_next = vp.tile([P, NT], F32, tag="mfM")
                    nc.vector.tensor_scalar(
                        out=mfM_next[:],
                        in0=kc_ps[:].rearrange("p (c two) -> p c two", two=2)[:, :, 0],
                        scalar1=BIGM / BIGS, scalar2=None, op0=OP.mult)
                prep = rp.tile([P, NP_], FP16, tag=f"prep_{br_tag}")
                nc.gpsimd.partition_broadcast(prep[:], pool[0:1, :])
                mrep = rp.tile([P, NP_], FP16, tag=f"mrep_{br_tag}")
                nc.gpsimd.partition_broadcast(mrep[:], mfS_next[0:1, :])
                nc.vector.tensor_tensor(out=out_tile[:], in0=hTf[:], in1=prep[:],
                                        op=OP.mult)
                hm = rp1.tile([P, NP_], BF16, tag=f"hm_{br_tag}")
                nc.vector.tensor_tensor(out=hm[:], in0=out_tile[:], in1=mrep[:],
                                        op=OP.add)
                mx = vp.tile([P, 1], BF16, tag="mx")
                nc.vector.tensor_reduce(out=mx[:], in_=hm[:], axis=AX.X, op=OP.max)
                nc.vector.tensor_tensor(out=gacc0[g][:], in0=gacc0[g][:],
                                        in1=mx[:], op=OP.add)
                mn = vp.tile([P, 1], BF16, tag="mn")
                with nc.allow_low_precision(reason="bf16 mean readout"):
                    nc.vector.tensor_reduce(out=mn[:], in_=out_tile[:], axis=AX.X,
                                            op=OP.add)
                nc.vector.tensor_scalar(out=mn[:], in0=mn[:], scalar1=1.0 / k,
                                        scalar2=None, op0=OP.mult)
                nc.vector.tensor_tensor(out=gacc1[g][:], in0=gacc1[g][:],
                                        in1=mn[:], op=OP.add)
                return mfS_next, mfM_next

            # ---- per-graph loop: wavefront schedule ----
            SKEW = int(os.environ.get("K_SKEW", 2))
            st = {}

            def load_graph(g):
                cnt_bf = dc.tile([P, NT * NP_], BF16, tag="cnt")
                for q in range(4):
                    qs = slice(q * 2 * NP_, (q + 1) * 2 * NP_)
                    nc.sync.dma_start(out=cnt_bf[:, qs], in_=cnt_d[g][:, qs])
                xT_b = dp.tile([P, NP_], BF16, tag="xT")
                nc.sync.dma_start(out=xT_b[:], in_=xT_d[g][:])
                xnm_b = dp.tile([P, NP_], BF16, tag="xnm")
                nc.sync.dma_start(out=xnm_b[:], in_=xnm_d[g][:])
                st[g] = dict(cnt=cnt_bf, hTb=xT_b, znm=xnm_b, zTb=xT_b,
                             zsub=xT_b, mfM=mfM0, mfS_g=mfS0, mfS_c=mfS0)

            def do_level(g, li):
                S = st[g]
                last = (li == 2)
                k = pools_gat[li][1]
                NO_GAT = os.environ.get("K_NO_GAT") == "1"
                NO_GC = os.environ.get("K_NO_GC") == "1"
                if not NO_GAT:
                    es2, hW_bf, ed_rep = gat_phase1(li, S["hTb"], S["mfM"])
                if not NO_GC:
                    aggT_bf = gc_agg(li, S["cnt"], S["znm"], S["zsub"])
                hTf = (gat_phase2(li, S["cnt"], es2, hW_bf, ed_rep)
                       if not NO_GAT else None)
                zTf = gc_out(li, aggT_bf, S["zTb"]) if not NO_GC else hTf
                if hTf is None:
                    hTf = zTf
                h_poolT = s3.tile([P, NP_], BF16, tag="hT")
                S["mfS_g"], mfM_n = topk_branch(
                    g, li, "g", hTf, S["mfS_g"], wp[pools_gat[li][0]], k,
                    h_poolT, need_col=not last)
                if mfM_n is not None:
                    S["mfM"] = mfM_n
                z_poolT = s3.tile([P, NP_], BF16, tag="zT")
                S["mfS_c"], _ = topk_branch(
                    g, li, "c", zTf, S["mfS_c"], wp[pools_gc[li][0]], k,
                    z_poolT, need_col=False)
                S["zsub"] = z_poolT
                if not last:
                    S["hTb"] = h_poolT
                    S["zTb"] = z_poolT
                    znm_b = sp.tile([P, NP_], BF16, tag="znm")
                    nc.sync.dma_start_transpose(
                        znm_b[:].rearrange("p (c d) -> p c d", d=P), z_poolT[:])
                    S["znm"] = znm_b

            events = []  # (wave, order, kind, g, li)
            for g in range(KG):
                events.append((g * SKEW - 1, 0, "load", g, 0))
                for li in range(3):
                    events.append((g * SKEW + li, 1, "level", g, li))
            events.sort(key=lambda e: (e[0], e[1], e[3]))
            for _, _, kind, g, li in events:
                if kind == "load":
                    load_graph(g)
                else:
                    do_level(g, li)

            # ---- MLP over all graphs ----
            t1_ps = psS.tile([P, 16], F32, tag="pcol")
            for g in range(G):
                nc.tensor.matmul(t1_ps[:, g:g + 1], Wl1a[:], gacc0[g][:],
                                 start=True, stop=False)
                nc.tensor.matmul(t1_ps[:, g:g + 1], Wl1b[:], gacc1[g][:],
                                 start=False, stop=True)
            t1 = cp.tile([P, G], BF16, tag="t1")
            nc.vector.tensor_scalar(out=t1[:], in0=t1_ps[:, 0:G], scalar1=bl1[:, 0:1],
                                    scalar2=0.0, op0=OP.add, op1=OP.max)
            t2_ps = psS.tile([P, 16], F32, tag="pcol")
            nc.tensor.matmul(t2_ps[0:64, 0:G], Wl2[:], t1[:], start=True, stop=True)
            t2p = cp.tile([64, G], BF16, tag="t2p")
            nc.vector.tensor_scalar(out=t2p[:], in0=t2_ps[0:64, 0:G], scalar1=bl2[:, 0:1],
                                    scalar2=None, op0=OP.add)
            t2 = cp.tile([64, G], BF16, tag="t2")
            nc.scalar.activation(out=t2[:], in_=t2p[:], func=AF.Prelu, alpha=0.01)
            t3_ps = psS.tile([P, 16], F32, tag="pcol")
            nc.tensor.matmul(t3_ps[0:C, 0:G], Wl3[:], t2[:], start=True, stop=True)
            lg_cm = cp.tile([C, G], F32, tag="lgcm")
            nc.vector.tensor_scalar(out=lg_cm[:], in0=t3_ps[0:C, 0:G], scalar1=bl3[:, 0:1],
                                    scalar2=None, op0=OP.add)
            lg_ps = psS.tile([P, 16], F32, tag="pcol")
            nc.tensor.matmul(lg_ps[0:G, 0:C], lg_cm[:], ident[0:C, 0:C],
                             is_transpose=True)
            lg = cp.tile([G, C], F32, tag="lg")
            nc.vector.tensor_copy(out=lg[:], in_=lg_ps[0:G, 0:C])
            ex = cp.tile([G, C], F32, tag="ex")
            nc.scalar.activation(out=ex[:], in_=lg[:], func=AF.Exp)
            S = cp.tile([G, 1], F32, tag="S")
            nc.vector.tensor_reduce(out=S[:], in_=ex[:], axis=AX.X, op=OP.add)
            # ln(S) via Newton: y += S*exp(-y) - 1
            y = cp.tile([G, 1], F32, tag="y")
            nc.vector.memset(y[:], 2.3)
            for _ in range(6):
                eny = cp.tile([G, 1], F32, tag="eny")
                nc.scalar.activation(out=eny[:], in_=y[:], func=AF.Exp, scale=-1.0)
                nc.vector.tensor_tensor(out=eny[:], in0=eny[:], in1=S[:], op=OP.mult)
                nc.vector.tensor_scalar(out=eny[:], in0=eny[:], scalar1=1.0,
                                        scalar2=None, op0=OP.subtract)
                nc.vector.tensor_tensor(out=y[:], in0=y[:], in1=eny[:], op=OP.add)
            outt = cp.tile([G, C], F32, tag="outt")
            nc.vector.tensor_scalar(out=outt[:], in0=lg[:], scalar1=y[:, 0:1],
                                    scalar2=None, op0=OP.subtract)
            nc.sync.dma_start(out=out_d[:], in_=outt[:])

    nc.compile()
    return nc


# ----------------------------------------------------------------------------
# host side
# ----------------------------------------------------------------------------

def _prep_in_maps(inputs):
    import ml_dtypes
    bf16 = ml_dtypes.bfloat16
    x = np.ascontiguousarray(np.asarray(inputs["x"], np.float32))
    ei = np.asarray(inputs["edge_index"]).astype(np.int64)
    src, dst = ei[0], ei[1]
    gid = src // NPG
    sl, dl = src % NPG, dst % NPG

    cnt = np.zeros((B, NP_, NP_), np.float32)
    np.add.at(cnt, (gid, sl, dl), 1.0)
    idx = np.arange(NP_)
    cnt[:, idx, idx] += 1.0  # GAT self-loops (GraphConv subtracts this back)
    # pack [g, src, dst] -> [g, p, t*1024+dst]
    cnt_pk = np.ascontiguousarray(
        cnt.reshape(B, NT, P, NP_).transpose(0, 2, 1, 3).reshape(B, P, NT * NP_)
    ).astype(bf16)

    x3 = x.reshape(B, NPG, D)
    x_pad = np.zeros((B, NP_, D), np.float32)
    x_pad[:, :NPG] = x3
    xT = np.ascontiguousarray(x_pad.transpose(0, 2, 1)).astype(bf16)  # [B,128,1024]
    xnm = np.ascontiguousarray(
        x_pad.reshape(B, NT, P, D).transpose(0, 2, 1, 3).reshape(B, P, NP_)
    ).astype(bf16)

    m0 = np.zeros((NP_,), np.float32)
    m0[:NPG] = 1.0
    mfM0 = np.ascontiguousarray(((m0 - 1.0) * BIGM).reshape(NT, P).T)  # [128, 8]
    mfS0 = np.zeros((33, NP_), np.float16)
    mfS0[0] = mfS0[32] = ((m0 - 1.0) * BIGS).astype(np.float16)

    def col(v):
        return np.ascontiguousarray(np.asarray(v, np.float32).reshape(-1, 1))

    weights = {}
    for l in (1, 2, 3):
        Wgl = np.asarray(inputs[f"W_g{l}"], np.float32)
        weights[f"W_g{l}"] = np.ascontiguousarray(Wgl).astype(bf16)
        weights[f"asd_g{l}"] = np.ascontiguousarray(
            Wgl @ np.stack([np.asarray(inputs[f"as_g{l}"], np.float32),
                            np.asarray(inputs[f"ad_g{l}"], np.float32)], axis=1)
        ).astype(bf16)
        weights[f"b_g{l}"] = col(inputs[f"b_g{l}"])
        weights[f"Wr_c{l}"] = np.ascontiguousarray(
            np.asarray(inputs[f"Wr_c{l}"], np.float32)).astype(bf16)
        weights[f"br_c{l}"] = col(inputs[f"br_c{l}"])
        weights[f"Wo_c{l}"] = np.ascontiguousarray(
            np.asarray(inputs[f"Wo_c{l}"], np.float32)).astype(bf16)
    for n in ("w_p20", "w_p30", "w_p11", "w_p21", "w_p31"):
        w = np.asarray(inputs[n], np.float32)
        weights[n] = col(w / np.linalg.norm(w)).astype(bf16)
    weights["W_l1"] = np.ascontiguousarray(np.asarray(inputs["W_l1"], np.float32)).astype(bf16)
    weights["b_l1"] = col(inputs["b_l1"])
    weights["W_l2"] = np.ascontiguousarray(np.asarray(inputs["W_l2"], np.float32)).astype(bf16)
    weights["b_l2"] = col(inputs["b_l2"])
    weights["W_l3"] = np.ascontiguousarray(np.asarray(inputs["W_l3"], np.float32)).astype(bf16)
    weights["b_l3"] = col(inputs["b_l3"])

    in_maps = []
    for c in range(NCORES):
        lo = c * G
        hi = min(lo + G, B)
        cs = np.zeros((G, P, NT * NP_), bf16)
        xs = np.zeros((G, P, NP_), bf16)
        xn = np.zeros((G, P, NP_), bf16)
        if hi > lo:
            cs[:hi - lo] = cnt_pk[lo:hi]
            xs[:hi - lo] = xT[lo:hi]
            xn[:hi - lo] = xnm[lo:hi]
        im = {"cnt_sh": cs, "xT_sh": xs, "xnm_sh": xn,
              "mfM0": mfM0, "mfS0": mfS0}
        im.update(weights)
        in_maps.append(im)
    return in_maps


def kernel(**inputs) -> np.ndarray:
    if "nc" not in _cache:
        _cache["nc"] = _build_program()
    nc = _cache["nc"]
    in_maps = _prep_in_maps(inputs)
    res = run_bass_kernel_spmd(nc, in_maps, list(range(NCORES)))
    out = np.zeros((B, C), np.float32)
    for c in range(NCORES):
        lo = c * G
        hi = min(lo + G, B)
        if hi > lo:
            out[lo:hi] = np.asarray(res.results[c]["out"])[:hi - lo]
    return out
